# revision 8
# baseline (speedup 1.0000x reference)
"""Additive attention (B=4, Q=512, K=2048, D=256, H=64) on 8 TRN2 NeuronCores.

Strategy
--------
scores[b,q,k] = sum_h wv[h] * tanh(qp[b,q,h] + kp[b,k,h]); masked softmax over k;
out = attn @ values.  The dominant cost is tanh on ACT (the only transcendental
engine, 1 elem/lane/cycle).  Structural optimizations:

1. Masked keys (k >= valid_len) contribute exp(-1e6) == 0 exactly, so only
   ceil(L_b/128)*128 key columns per batch need any compute.  The host gathers
   just the valid key-chunks; the graph is specialized at runtime to the
   valid_lens actually received (all 8 cores share one graph; per-core
   variation is data only).

2. Layout h-on-partitions with TWO queries packed per 128-partition ACT call
   (partitions 0:64 = h for q-slot-A, 64:128 = h for q-slot-B, against a kp2
   tile holding kp.T duplicated in both halves).  The "+qp" broadcast add is
   free via ACT's per-partition bias operand.  The h-reduction runs on the PE:
   pair j multiplies with a (128,32) weight whose only nonzero columns are 2j
   (top-half wv) and 2j+1 (bottom-half wv), accumulating into a shared PSUM
   tile -> 16 pairs build 32 score rows at no extra PE streaming cost.

3. All matmul operands are bf16 (fp32 streams at half rate on the PE; PSUM
   accumulation stays fp32).  ACT is dtype-independent so tanh/exp lose
   nothing; precision impact ~1e-3 total.

Softmax needs no max-subtraction (|score| <= ||wv||_1 ~ 2.6) and no on-device
normalization: an appended ones-column in the values matrix yields sum(exp)
via the same value matmul, and the host divides.  Each query lives on exactly
one core (core c owns q[64c:64c+64) of every batch), so no merge is needed.
"""

import math

import numpy as np

B, Q, KK, D, H = 4, 512, 2048, 256, 64
P = 128
CH = 128          # key-chunk granularity
QPC = 64          # queries per (core, batch)
NCORES = 8
DPAD = D + 8      # values cols + [ones, 7*zero] padding

_GRAPH_CACHE: dict = {}


def _build_graph(nbs):
    """Build + compile the single-core SPMD graph for per-batch chunk counts nbs."""
    import concourse.bass as bass
    import concourse.mybir as mybir
    from concourse import bacc
    from concourse.tile import TileContext

    f32 = mybir.dt.float32
    bf16 = mybir.dt.bfloat16
    AF = mybir.ActivationFunctionType
    C = sum(nbs) * CH
    assert C > 0
    offs = np.concatenate([[0], np.cumsum(np.asarray(nbs) * CH)]).astype(int)
    totch = C // CH

    nc = bacc.Bacc("TRN2", target_bir_lowering=False, debug=False)
    qT_e = nc.declare_dram_parameter("qT", [D, 4 * QPC], bf16, isOutput=False)
    kT_e = nc.declare_dram_parameter("keysT", [D, C], bf16, isOutput=False)
    va_e = nc.declare_dram_parameter("vaug", [C, DPAD], bf16, isOutput=False)
    wq_e = nc.declare_dram_parameter("Wq", [D, H], bf16, isOutput=False)
    wk2_e = nc.declare_dram_parameter("Wk2", [D, P], bf16, isOutput=False)
    wvs_e = nc.declare_dram_parameter("wv2s", [P, 16, 32], bf16, isOutput=False)
    id_e = nc.declare_dram_parameter("ident", [P, P], bf16, isOutput=False)
    out_e = nc.declare_dram_parameter("out", [DPAD, 4 * QPC], f32, isOutput=True)

    with TileContext(nc) as tc:
        with (
            tc.tile_pool(name="const", bufs=1) as cpool,
            tc.tile_pool(name="big", bufs=1) as kpool,
            tc.tile_pool(name="feat", bufs=3) as fpool,
            tc.tile_pool(name="pexp", bufs=2) as ppool,
            tc.tile_pool(name="pts", bufs=4) as ptpool,
            tc.tile_pool(name="stage", bufs=1) as stpool,
        ):
            # ---- constant / input loads
            id_t = cpool.tile([P, P], bf16)
            nc.sync.dma_start(id_t[:], id_e[:])
            wvs_t = cpool.tile([P, 16, 32], bf16)
            nc.sync.dma_start(wvs_t[:], wvs_e[:])
            wq_t = cpool.tile([P, 2, H], bf16)
            nc.sync.dma_start(wq_t[:], wq_e[:].rearrange("(c p) h -> p c h", p=P))
            wk2_t = cpool.tile([P, 2, P], bf16)
            nc.sync.dma_start(wk2_t[:], wk2_e[:].rearrange("(c p) h -> p c h", p=P))
            qt_t = cpool.tile([P, 2, 4 * QPC], bf16)
            nc.sync.dma_start(qt_t[:], qT_e[:].rearrange("(c p) q -> p c q", p=P))
            kt0 = kpool.tile([P, C], bf16)
            kt1 = kpool.tile([P, C], bf16)
            va_t = kpool.tile([P, totch, DPAD], bf16)
            nc.sync.dma_start(va_t[:], va_e[:].rearrange("(n p) d -> p n d", p=P))

            # ---- kp2 = Wk2.T @ keysT  -> (128, C) in SBUF (both halves = kp.T)
            # keysT is DMA'd and projected in 512-col slices so the first
            # tanh can start as soon as region 0's columns land.
            kp2 = kpool.tile([P, C], bf16)
            qp2 = cpool.tile([P, QPC // 2 * 4], f32)
            with tc.tile_pool(name="ps_mm", bufs=2, space="PSUM") as ps_mm:
                # qp2 first: tiny DMA + matmuls; bias must be ready for tanh #1
                qps = ps_mm.tile([H, 4 * QPC], f32, tag="qp")
                nc.tensor.matmul(qps[:], lhsT=wq_t[:, 0, :], rhs=qt_t[:, 0, :],
                                 start=True, stop=False)
                nc.tensor.matmul(qps[:], lhsT=wq_t[:, 1, :], rhs=qt_t[:, 1, :],
                                 start=False, stop=True)
                for b in range(B):
                    if nbs[b] == 0:
                        continue
                    nc.vector.tensor_copy(qp2[0:H, 32 * b:32 * b + 32],
                                          qps[:, QPC * b:QPC * b + 32])
                    nc.vector.tensor_copy(qp2[H:P, 32 * b:32 * b + 32],
                                          qps[:, QPC * b + 32:QPC * b + 64])

                for c0 in range(0, C, 512):
                    w = min(512, C - c0)
                    nc.sync.dma_start(kt0[:, c0:c0 + w], kT_e[0:P, c0:c0 + w])
                    nc.sync.dma_start(kt1[:, c0:c0 + w], kT_e[P:D, c0:c0 + w])
                    pt = ps_mm.tile([P, 512], f32, tag="kp")
                    nc.tensor.matmul(pt[:, :w], lhsT=wk2_t[:, 0, :],
                                     rhs=kt0[:, c0:c0 + w], start=True, stop=False)
                    nc.tensor.matmul(pt[:, :w], lhsT=wk2_t[:, 1, :],
                                     rhs=kt1[:, c0:c0 + w], start=False, stop=True)
                    nc.vector.tensor_copy(kp2[:, c0:c0 + w], pt[:, :w])

            st0 = stpool.tile([P, 4 * QPC], f32)
            st1 = stpool.tile([P, 4 * QPC], f32)
            st2 = stpool.tile([8, 4 * QPC], f32)

            # ---- per-batch regions
            with (
                tc.tile_pool(name="ps_sc", bufs=1, space="PSUM") as ps_sc,
                tc.tile_pool(name="ps_tr", bufs=2, space="PSUM") as ps_tr,
                tc.tile_pool(name="ps_v", bufs=1, space="PSUM") as ps_v,
            ):
                for b in range(B):
                    nb = nbs[b]
                    if nb == 0:
                        continue
                    W = nb * CH
                    off = int(offs[b])
                    sc = ps_sc.tile([QPC, W], f32, tag="sc")
                    for g in range(2):
                        for j in range(16):
                            ft = fpool.tile([P, W], bf16, tag="ft")
                            nc.scalar.activation(
                                ft[:], kp2[:, off:off + W], AF.Tanh,
                                bias=qp2[:, 32 * b + 16 * g + j:32 * b + 16 * g + j + 1],
                                scale=1.0)
                            for c0 in range(0, W, 512):
                                w = min(512, W - c0)
                                nc.tensor.matmul(sc[32 * g:32 * g + 32, c0:c0 + w],
                                                 lhsT=wvs_t[:, j, :], rhs=ft[:, c0:c0 + w],
                                                 start=(j == 0), stop=(j == 15))
                    pe = ppool.tile([QPC, W], bf16, tag="P")
                    nc.scalar.activation(pe[:], sc[:], AF.Exp)
                    # one PSUM bank holds all value accumulators:
                    # cols 0:64 = values[:,0:128].T @ P, 64:128 = values[:,128:256].T @ P,
                    # 128:192 (partitions 0:8) = [ones|pad].T @ P
                    vp = ps_v.tile([P, 3 * QPC], f32, tag="v")
                    for ci in range(nb):
                        tp = ps_tr.tile([P, QPC], bf16, tag="tr")
                        nc.tensor.transpose(tp[:], pe[:, CH * ci:CH * ci + CH],
                                            id_t[0:QPC, 0:QPC])
                        ptt = ptpool.tile([P, QPC], bf16, tag="pt")
                        nc.vector.tensor_copy(ptt[:], tp[:])
                        gi = off // CH + ci
                        # start=True clears has_written for the WHOLE bank, so
                        # only the very first matmul into this bank may use it;
                        # later first-writes rely on overwrite-where-bit-clear.
                        st, sp = (ci == 0), (ci == nb - 1)
                        nc.tensor.matmul(vp[:, 0:QPC], lhsT=va_t[:, gi, 0:128],
                                         rhs=ptt[:], start=st, stop=sp,
                                         skip_group_check=True)
                        nc.tensor.matmul(vp[:, QPC:2 * QPC], lhsT=va_t[:, gi, 128:256],
                                         rhs=ptt[:], start=False, stop=sp,
                                         skip_group_check=True)
                        nc.tensor.matmul(vp[0:8, 2 * QPC:3 * QPC],
                                         lhsT=va_t[:, gi, 256:264],
                                         rhs=ptt[:], start=False, stop=sp,
                                         skip_group_check=True)
                    nc.vector.tensor_copy(st0[:, QPC * b:QPC * (b + 1)], vp[:, 0:QPC])
                    nc.vector.tensor_copy(st1[:, QPC * b:QPC * (b + 1)],
                                          vp[:, QPC:2 * QPC])
                    nc.vector.tensor_copy(st2[:, QPC * b:QPC * (b + 1)],
                                          vp[0:8, 2 * QPC:3 * QPC])
                    # stream this region's output slice immediately
                    cs = slice(QPC * b, QPC * (b + 1))
                    nc.sync.dma_start(out_e[0:128, cs], st0[:, cs])
                    nc.sync.dma_start(out_e[128:256, cs], st1[:, cs])
                    nc.sync.dma_start(out_e[256:264, cs], st2[:, cs])

    nc.compile()
    return nc


def _prep_inputs(queries, keys, values, L, Wq, Wk, wv, nbs):
    import ml_dtypes
    bf = ml_dtypes.bfloat16
    C = sum(nbs) * CH
    keysT = np.zeros((D, C), np.float32)
    vaug = np.zeros((C, DPAD), np.float32)
    off = 0
    for b in range(B):
        nb = nbs[b]
        if nb == 0:
            continue
        n = nb * CH
        lb = int(L[b])
        kb = keys[b, :n, :].copy()
        vb = values[b, :n, :].copy()
        kb[lb:] = 0.0
        vb[lb:] = 0.0
        keysT[:, off:off + n] = kb.T
        vaug[off:off + n, 0:D] = vb
        vaug[off:off + n, D] = (np.arange(n) < lb).astype(np.float32)
        off += n
    wv2s = np.zeros((P, 16, 32), np.float32)
    for j in range(16):
        wv2s[0:H, j, 2 * j] = wv
        wv2s[H:P, j, 2 * j + 1] = wv
    Wk2 = np.concatenate([Wk, Wk], axis=1)
    ident = np.eye(P, dtype=np.float32)
    keysT = keysT.astype(bf)
    vaug = vaug.astype(bf)
    wv2s = wv2s.astype(bf)
    Wk2 = np.ascontiguousarray(Wk2.astype(bf))
    Wq_b = Wq.astype(bf)
    ident = ident.astype(bf)
    in_maps = []
    for c in range(NCORES):
        qT = np.zeros((D, 4 * QPC), np.float32)
        for b in range(B):
            qT[:, QPC * b:QPC * (b + 1)] = queries[b, c * QPC:(c + 1) * QPC, :].T
        in_maps.append(dict(qT=qT.astype(bf), keysT=keysT, vaug=vaug, Wq=Wq_b,
                            Wk2=Wk2, wv2s=wv2s, ident=ident))
    return in_maps


def kernel(queries, keys, values, valid_lens, Wq, Wk, wv):
    from concourse.bass_utils import run_bass_kernel_spmd

    queries = np.ascontiguousarray(np.asarray(queries, dtype=np.float32))
    keys = np.ascontiguousarray(np.asarray(keys, dtype=np.float32))
    values = np.ascontiguousarray(np.asarray(values, dtype=np.float32))
    Wq = np.ascontiguousarray(np.asarray(Wq, dtype=np.float32))
    Wk = np.ascontiguousarray(np.asarray(Wk, dtype=np.float32))
    wv = np.ascontiguousarray(np.asarray(wv, dtype=np.float32))
    L = np.clip(np.asarray(valid_lens).astype(np.int64), 0, KK)
    nbs = tuple(int(math.ceil(int(l) / CH)) for l in L)

    out = np.zeros((B, Q, D), np.float32)
    for b in range(B):
        if nbs[b] == 0:
            # softmax over all -1e6 scores is uniform over ALL keys
            out[b, :, :] = values[b].mean(axis=0)[None, :]

    if sum(nbs) > 0:
        if nbs not in _GRAPH_CACHE:
            _GRAPH_CACHE[nbs] = _build_graph(nbs)
        nc = _GRAPH_CACHE[nbs]
        in_maps = _prep_inputs(queries, keys, values, L, Wq, Wk, wv, nbs)
        res = run_bass_kernel_spmd(nc, in_maps, list(range(NCORES)))
        perm = np.array([32 * (r % 2) + r // 2 for r in range(QPC)])
        for c in range(NCORES):
            o = res.results[c]["out"]  # (DPAD, 256)
            for b in range(B):
                if nbs[b] == 0:
                    continue
                blk = o[:, QPC * b:QPC * (b + 1)]
                vals = blk[0:D, :]          # (256, 64) = out.T, permuted cols
                sums = blk[D, :]            # (64,)
                out[b, c * QPC + perm, :] = (vals / sums[None, :]).T
    return out


# revision 16
# speedup vs baseline: 1.0244x; 1.0244x over previous
"""Additive attention (B=4, Q=512, K=2048, D=256, H=64) on 8 TRN2 NeuronCores.

Strategy
--------
scores[b,q,k] = sum_h wv[h] * tanh(qp[b,q,h] + kp[b,k,h]); masked softmax over k;
out = attn @ values.  The dominant cost is tanh on ACT (the only transcendental
engine, 1 elem/lane/cycle).  Structural optimizations:

1. Masked keys (k >= valid_len) contribute exp(-1e6) == 0 exactly, so only
   ceil(L_b/128)*128 key columns per batch need any compute.  The host gathers
   just the valid key-chunks; the graph is specialized at runtime to the
   valid_lens actually received (all 8 cores share one graph; per-core
   variation is data only).

2. Layout h-on-partitions with TWO queries packed per 128-partition ACT call
   (partitions 0:64 = h for q-slot-A, 64:128 = h for q-slot-B, against a kp2
   tile holding kp.T duplicated in both halves).  The "+qp" broadcast add is
   free via ACT's per-partition bias operand.  The h-reduction runs on the PE:
   pair j multiplies with a (128,32) weight whose only nonzero columns are 2j
   (top-half wv) and 2j+1 (bottom-half wv), accumulating into a shared PSUM
   tile -> 16 pairs build 32 score rows at no extra PE streaming cost.

3. All matmul operands are bf16 (fp32 streams at half rate on the PE; PSUM
   accumulation stays fp32).  ACT is dtype-independent so tanh/exp lose
   nothing; precision impact ~1e-3 total.

Softmax needs no max-subtraction (|score| <= ||wv||_1 ~ 2.6) and no on-device
normalization: an appended ones-column in the values matrix yields sum(exp)
via the same value matmul, and the host divides.  Each query lives on exactly
one core (core c owns q[64c:64c+64) of every batch), so no merge is needed.
"""

import math

import numpy as np

B, Q, KK, D, H = 4, 512, 2048, 256, 64
P = 128
CH = 128          # key-chunk granularity
QPC = 64          # queries per (core, batch)
NCORES = 8
DPAD = D + 8      # values cols + [ones, 7*zero] padding

_GRAPH_CACHE: dict = {}


def _build_graph(nbs):
    """Build + compile the single-core SPMD graph for per-batch chunk counts nbs."""
    import concourse.bass as bass
    import concourse.mybir as mybir
    from concourse import bacc
    from concourse.tile import TileContext

    f32 = mybir.dt.float32
    bf16 = mybir.dt.bfloat16
    AF = mybir.ActivationFunctionType
    C = sum(nbs) * CH
    assert C > 0
    offs = np.concatenate([[0], np.cumsum(np.asarray(nbs) * CH)]).astype(int)
    totch = C // CH

    nc = bacc.Bacc("TRN2", target_bir_lowering=False, debug=False)
    # wqk2: cols 0:64 = Wq, 64:192 = [Wk|Wk]; wvsid: cols 0:512 = wv2s flat,
    # 512:640 = identity.  Combined tensors keep the serialized per-DMA issue
    # cost (~0.6us each) off the critical path.
    qT_e = nc.declare_dram_parameter("qT", [D, 4 * QPC], bf16, isOutput=False)
    kT_e = nc.declare_dram_parameter("keysT", [D, C], bf16, isOutput=False)
    va_e = nc.declare_dram_parameter("vaug", [C, DPAD], bf16, isOutput=False)
    wqk2_e = nc.declare_dram_parameter("wqk2", [D, H + P], bf16, isOutput=False)
    wvsid_e = nc.declare_dram_parameter("wvsid", [P, 16 * 32 + P], bf16,
                                        isOutput=False)
    out_e = nc.declare_dram_parameter("out", [P, 4 * 3 * QPC], f32, isOutput=True)

    with TileContext(nc) as tc:
        with (
            tc.tile_pool(name="const", bufs=1) as cpool,
            tc.tile_pool(name="big", bufs=1) as kpool,
            tc.tile_pool(name="feat", bufs=3) as fpool,
            tc.tile_pool(name="pexp", bufs=2) as ppool,
            tc.tile_pool(name="pts", bufs=4) as ptpool,
        ):
            # ---- input loads, critical-path first
            wqk2_t = cpool.tile([P, 2, H + P], bf16)
            nc.sync.dma_start(wqk2_t[:], wqk2_e[:].rearrange("(c p) h -> p c h", p=P))
            qt_t = cpool.tile([P, 2, 4 * QPC], bf16)
            nc.sync.dma_start(qt_t[:], qT_e[:].rearrange("(c p) q -> p c q", p=P))
            kt_t = kpool.tile([P, 2, C], bf16)
            nc.sync.dma_start(kt_t[:], kT_e[:].rearrange("(c p) k -> p c k", p=P))
            wvs_t = cpool.tile([P, 16, 32], bf16)
            nc.sync.dma_start(wvs_t[:], wvsid_e[:, 0:512].rearrange(
                "p (j c) -> p j c", j=16))
            id_t = cpool.tile([P, P], bf16)
            nc.sync.dma_start(id_t[:], wvsid_e[:, 512:512 + P])
            va_t = kpool.tile([P, totch, DPAD], bf16)
            nc.sync.dma_start(va_t[:], va_e[:].rearrange("(n p) d -> p n d", p=P))

            # ---- kp2 = Wk2.T @ keysT  -> (128, C) in SBUF (both halves = kp.T)
            kp2 = kpool.tile([P, C], bf16)
            qp2 = cpool.tile([P, QPC // 2 * 4], f32)
            with tc.tile_pool(name="ps_mm", bufs=2, space="PSUM") as ps_mm:
                # qp2 first: tiny DMA + matmuls; bias must be ready for tanh #1
                qps = ps_mm.tile([H, 4 * QPC], f32, tag="qp")
                nc.tensor.matmul(qps[:], lhsT=wqk2_t[:, 0, 0:H], rhs=qt_t[:, 0, :],
                                 start=True, stop=False)
                nc.tensor.matmul(qps[:], lhsT=wqk2_t[:, 1, 0:H], rhs=qt_t[:, 1, :],
                                 start=False, stop=True)
                for b in range(B):
                    if nbs[b] == 0:
                        continue
                    nc.vector.tensor_copy(qp2[0:H, 32 * b:32 * b + 32],
                                          qps[:, QPC * b:QPC * b + 32])
                    nc.vector.tensor_copy(qp2[H:P, 32 * b:32 * b + 32],
                                          qps[:, QPC * b + 32:QPC * b + 64])

                for c0 in range(0, C, 512):
                    w = min(512, C - c0)
                    pt = ps_mm.tile([P, 512], f32, tag="kp")
                    nc.tensor.matmul(pt[:, :w], lhsT=wqk2_t[:, 0, H:H + P],
                                     rhs=kt_t[:, 0, c0:c0 + w], start=True, stop=False)
                    nc.tensor.matmul(pt[:, :w], lhsT=wqk2_t[:, 1, H:H + P],
                                     rhs=kt_t[:, 1, c0:c0 + w], start=False, stop=True)
                    nc.vector.tensor_copy(kp2[:, c0:c0 + w], pt[:, :w])

            # ---- per-batch regions
            with (
                tc.tile_pool(name="ps_sc", bufs=1, space="PSUM") as ps_sc,
                tc.tile_pool(name="ps_tr", bufs=2, space="PSUM") as ps_tr,
                tc.tile_pool(name="ps_v", bufs=1, space="PSUM") as ps_v,
            ):
                for b in range(B):
                    nb = nbs[b]
                    if nb == 0:
                        continue
                    W = nb * CH
                    off = int(offs[b])
                    sc = ps_sc.tile([QPC, W], f32, tag="sc")
                    for g in range(2):
                        for j in range(16):
                            ft = fpool.tile([P, W], bf16, tag="ft")
                            nc.scalar.activation(
                                ft[:], kp2[:, off:off + W], AF.Tanh,
                                bias=qp2[:, 32 * b + 16 * g + j:32 * b + 16 * g + j + 1],
                                scale=1.0)
                            for c0 in range(0, W, 512):
                                w = min(512, W - c0)
                                nc.tensor.matmul(sc[32 * g:32 * g + 32, c0:c0 + w],
                                                 lhsT=wvs_t[:, j, :], rhs=ft[:, c0:c0 + w],
                                                 start=(j == 0), stop=(j == 15))
                    pe = ppool.tile([QPC, W], bf16, tag="P")
                    nc.scalar.activation(pe[:], sc[:], AF.Exp)
                    # one PSUM bank holds all value accumulators:
                    # cols 0:64 = values[:,0:128].T @ P, 64:128 = values[:,128:256].T @ P,
                    # 128:192 (partitions 0:8) = [ones|pad].T @ P
                    vp = ps_v.tile([P, 3 * QPC], f32, tag="v")
                    for ci in range(nb):
                        tp = ps_tr.tile([P, QPC], bf16, tag="tr")
                        nc.tensor.transpose(tp[:], pe[:, CH * ci:CH * ci + CH],
                                            id_t[0:QPC, 0:QPC])
                        ptt = ptpool.tile([P, QPC], bf16, tag="pt")
                        nc.vector.tensor_copy(ptt[:], tp[:])
                        gi = off // CH + ci
                        # start=True clears has_written for the WHOLE bank, so
                        # only the very first matmul into this bank may use it;
                        # later first-writes rely on overwrite-where-bit-clear.
                        st, sp = (ci == 0), (ci == nb - 1)
                        nc.tensor.matmul(vp[:, 0:QPC], lhsT=va_t[:, gi, 0:128],
                                         rhs=ptt[:], start=st, stop=sp,
                                         skip_group_check=True)
                        nc.tensor.matmul(vp[:, QPC:2 * QPC], lhsT=va_t[:, gi, 128:256],
                                         rhs=ptt[:], start=False, stop=sp,
                                         skip_group_check=True)
                        nc.tensor.matmul(vp[0:8, 2 * QPC:3 * QPC],
                                         lhsT=va_t[:, gi, 256:264],
                                         rhs=ptt[:], start=False, stop=sp,
                                         skip_group_check=True)
                    # stream this region's whole accumulator out as one DMA
                    stg = ptpool.tile([P, 3 * QPC], f32, tag="stg")
                    nc.vector.tensor_copy(stg[:], vp[:])
                    nc.sync.dma_start(out_e[:, 3 * QPC * b:3 * QPC * (b + 1)], stg[:])

    nc.compile()
    return nc


def _prep_inputs(queries, keys, values, L, Wq, Wk, wv, nbs):
    import ml_dtypes
    bf = ml_dtypes.bfloat16
    C = sum(nbs) * CH
    keysT = np.zeros((D, C), np.float32)
    vaug = np.zeros((C, DPAD), np.float32)
    off = 0
    for b in range(B):
        nb = nbs[b]
        if nb == 0:
            continue
        n = nb * CH
        lb = int(L[b])
        kb = keys[b, :n, :].copy()
        vb = values[b, :n, :].copy()
        kb[lb:] = 0.0
        vb[lb:] = 0.0
        keysT[:, off:off + n] = kb.T
        vaug[off:off + n, 0:D] = vb
        vaug[off:off + n, D] = (np.arange(n) < lb).astype(np.float32)
        off += n
    wv2s = np.zeros((P, 16, 32), np.float32)
    for j in range(16):
        wv2s[0:H, j, 2 * j] = wv
        wv2s[H:P, j, 2 * j + 1] = wv
    wqk2 = np.concatenate([Wq, Wk, Wk], axis=1)            # (256, 192)
    wvsid = np.concatenate([wv2s.reshape(P, 512), np.eye(P, dtype=np.float32)],
                           axis=1)                          # (128, 640)
    keysT = keysT.astype(bf)
    vaug = vaug.astype(bf)
    wqk2 = np.ascontiguousarray(wqk2.astype(bf))
    wvsid = np.ascontiguousarray(wvsid.astype(bf))
    in_maps = []
    for c in range(NCORES):
        qT = np.zeros((D, 4 * QPC), np.float32)
        for b in range(B):
            qT[:, QPC * b:QPC * (b + 1)] = queries[b, c * QPC:(c + 1) * QPC, :].T
        in_maps.append(dict(qT=qT.astype(bf), keysT=keysT, vaug=vaug,
                            wqk2=wqk2, wvsid=wvsid))
    return in_maps


def kernel(queries, keys, values, valid_lens, Wq, Wk, wv):
    from concourse.bass_utils import run_bass_kernel_spmd

    queries = np.ascontiguousarray(np.asarray(queries, dtype=np.float32))
    keys = np.ascontiguousarray(np.asarray(keys, dtype=np.float32))
    values = np.ascontiguousarray(np.asarray(values, dtype=np.float32))
    Wq = np.ascontiguousarray(np.asarray(Wq, dtype=np.float32))
    Wk = np.ascontiguousarray(np.asarray(Wk, dtype=np.float32))
    wv = np.ascontiguousarray(np.asarray(wv, dtype=np.float32))
    L = np.clip(np.asarray(valid_lens).astype(np.int64), 0, KK)
    nbs = tuple(int(math.ceil(int(l) / CH)) for l in L)

    out = np.zeros((B, Q, D), np.float32)
    for b in range(B):
        if nbs[b] == 0:
            # softmax over all -1e6 scores is uniform over ALL keys
            out[b, :, :] = values[b].mean(axis=0)[None, :]

    if sum(nbs) > 0:
        if nbs not in _GRAPH_CACHE:
            _GRAPH_CACHE[nbs] = _build_graph(nbs)
        nc = _GRAPH_CACHE[nbs]
        in_maps = _prep_inputs(queries, keys, values, L, Wq, Wk, wv, nbs)
        res = run_bass_kernel_spmd(nc, in_maps, list(range(NCORES)))
        perm = np.array([32 * (r % 2) + r // 2 for r in range(QPC)])
        for c in range(NCORES):
            o = res.results[c]["out"]  # (128, 4*192): per region [d0|d1|sums]
            for b in range(B):
                if nbs[b] == 0:
                    continue
                blk = o[:, 3 * QPC * b:3 * QPC * (b + 1)]   # (128, 192)
                vals = np.concatenate([blk[:, 0:QPC], blk[:, QPC:2 * QPC]],
                                      axis=0)                # (256, 64)
                sums = blk[0, 2 * QPC:3 * QPC]               # (64,)
                out[b, c * QPC + perm, :] = (vals / sums[None, :]).T
    return out


# revision 17
# speedup vs baseline: 1.0454x; 1.0205x over previous
"""Additive attention (B=4, Q=512, K=2048, D=256, H=64) on 8 TRN2 NeuronCores.

Strategy
--------
scores[b,q,k] = sum_h wv[h] * tanh(qp[b,q,h] + kp[b,k,h]); masked softmax over k;
out = attn @ values.  The dominant cost is tanh on ACT (the only transcendental
engine, 1 elem/lane/cycle).  Structural optimizations:

1. Masked keys (k >= valid_len) contribute exp(-1e6) == 0 exactly, so only
   ceil(L_b/128)*128 key columns per batch need any compute.  The host gathers
   just the valid key-chunks; the graph is specialized at runtime to the
   valid_lens actually received (all 8 cores share one graph; per-core
   variation is data only).

2. Layout h-on-partitions with TWO queries packed per 128-partition ACT call
   (partitions 0:64 = h for q-slot-A, 64:128 = h for q-slot-B, against a kp2
   tile holding kp.T duplicated in both halves).  The "+qp" broadcast add is
   free via ACT's per-partition bias operand.  The h-reduction runs on the PE:
   pair j multiplies with a (128,32) weight whose only nonzero columns are 2j
   (top-half wv) and 2j+1 (bottom-half wv), accumulating into a shared PSUM
   tile -> 16 pairs build 32 score rows at no extra PE streaming cost.

3. All matmul operands are bf16 (fp32 streams at half rate on the PE; PSUM
   accumulation stays fp32).  ACT is dtype-independent so tanh/exp lose
   nothing; precision impact ~1e-3 total.

Softmax needs no max-subtraction (|score| <= ||wv||_1 ~ 2.6) and no on-device
normalization: an appended ones-column in the values matrix yields sum(exp)
via the same value matmul, and the host divides.  Each query lives on exactly
one core (core c owns q[64c:64c+64) of every batch), so no merge is needed.
"""

import math

import numpy as np

B, Q, KK, D, H = 4, 512, 2048, 256, 64
P = 128
CH = 128          # key-chunk granularity
QPC = 64          # queries per (core, batch)
NCORES = 8
DPAD = D + 8      # values cols + [ones, 7*zero] padding

_GRAPH_CACHE: dict = {}


def _build_graph(nbs):
    """Build + compile the single-core SPMD graph for per-batch chunk counts nbs."""
    import concourse.bass as bass
    import concourse.mybir as mybir
    from concourse import bacc
    from concourse.tile import TileContext

    f32 = mybir.dt.float32
    bf16 = mybir.dt.bfloat16
    AF = mybir.ActivationFunctionType
    C = sum(nbs) * CH
    assert C > 0
    offs = np.concatenate([[0], np.cumsum(np.asarray(nbs) * CH)]).astype(int)
    totch = C // CH

    nc = bacc.Bacc("TRN2", target_bir_lowering=False, debug=False)
    # wqk2: cols 0:64 = Wq, 64:192 = [Wk|Wk]; wvsid: cols 0:512 = wv2s flat,
    # 512:640 = identity.  Combined tensors keep the serialized per-DMA issue
    # cost (~0.6us each) off the critical path.
    qT_e = nc.declare_dram_parameter("qT", [D, 4 * QPC], bf16, isOutput=False)
    kT_e = nc.declare_dram_parameter("keysT", [D, C], bf16, isOutput=False)
    va_e = nc.declare_dram_parameter("vaug", [C, DPAD], bf16, isOutput=False)
    wqk2_e = nc.declare_dram_parameter("wqk2", [D, H + P], bf16, isOutput=False)
    wvsid_e = nc.declare_dram_parameter("wvsid", [P, 16 * 32 + P], bf16,
                                        isOutput=False)
    out_e = nc.declare_dram_parameter("out", [P, 4 * 3 * QPC], f32, isOutput=True)

    with TileContext(nc) as tc:
        with (
            tc.tile_pool(name="const", bufs=1) as cpool,
            tc.tile_pool(name="big", bufs=1) as kpool,
            tc.tile_pool(name="feat", bufs=6) as fpool,
            tc.tile_pool(name="pexp", bufs=2) as ppool,
            tc.tile_pool(name="pts", bufs=4) as ptpool,
        ):
            # ---- input loads, critical-path first
            wqk2_t = cpool.tile([P, 2, H + P], bf16)
            nc.sync.dma_start(wqk2_t[:], wqk2_e[:].rearrange("(c p) h -> p c h", p=P))
            qt_t = cpool.tile([P, 2, 4 * QPC], bf16)
            nc.sync.dma_start(qt_t[:], qT_e[:].rearrange("(c p) q -> p c q", p=P))
            kt_t = kpool.tile([P, 2, C], bf16)
            nc.sync.dma_start(kt_t[:], kT_e[:].rearrange("(c p) k -> p c k", p=P))
            wvs_t = cpool.tile([P, 16, 32], bf16)
            nc.sync.dma_start(wvs_t[:], wvsid_e[:, 0:512].rearrange(
                "p (j c) -> p j c", j=16))
            id_t = cpool.tile([P, P], bf16)
            nc.sync.dma_start(id_t[:], wvsid_e[:, 512:512 + P])
            va_t = kpool.tile([P, totch, DPAD], bf16)
            nc.sync.dma_start(va_t[:], va_e[:].rearrange("(n p) d -> p n d", p=P))

            # ---- kp2 = Wk2.T @ keysT  -> (128, C) in SBUF (both halves = kp.T)
            kp2 = kpool.tile([P, C], bf16)
            qp2 = cpool.tile([P, QPC // 2 * 4], f32)
            with tc.tile_pool(name="ps_mm", bufs=2, space="PSUM") as ps_mm:
                # qp2 first: tiny DMA + matmuls; bias must be ready for tanh #1
                qps = ps_mm.tile([H, 4 * QPC], f32, tag="qp")
                nc.tensor.matmul(qps[:], lhsT=wqk2_t[:, 0, 0:H], rhs=qt_t[:, 0, :],
                                 start=True, stop=False)
                nc.tensor.matmul(qps[:], lhsT=wqk2_t[:, 1, 0:H], rhs=qt_t[:, 1, :],
                                 start=False, stop=True)
                for b in range(B):
                    if nbs[b] == 0:
                        continue
                    nc.vector.tensor_copy(qp2[0:H, 32 * b:32 * b + 32],
                                          qps[:, QPC * b:QPC * b + 32])
                    nc.vector.tensor_copy(qp2[H:P, 32 * b:32 * b + 32],
                                          qps[:, QPC * b + 32:QPC * b + 64])

                for c0 in range(0, C, 512):
                    w = min(512, C - c0)
                    pt = ps_mm.tile([P, 512], f32, tag="kp")
                    nc.tensor.matmul(pt[:, :w], lhsT=wqk2_t[:, 0, H:H + P],
                                     rhs=kt_t[:, 0, c0:c0 + w], start=True, stop=False)
                    nc.tensor.matmul(pt[:, :w], lhsT=wqk2_t[:, 1, H:H + P],
                                     rhs=kt_t[:, 1, c0:c0 + w], start=False, stop=True)
                    nc.vector.tensor_copy(kp2[:, c0:c0 + w], pt[:, :w])

            # ---- per-batch regions
            with (
                tc.tile_pool(name="ps_sc", bufs=1, space="PSUM") as ps_sc,
                tc.tile_pool(name="ps_tr", bufs=2, space="PSUM") as ps_tr,
                tc.tile_pool(name="ps_v", bufs=1, space="PSUM") as ps_v,
            ):
                for b in range(B):
                    nb = nbs[b]
                    if nb == 0:
                        continue
                    W = nb * CH
                    off = int(offs[b])
                    sc = ps_sc.tile([QPC, W], f32, tag="sc")
                    for g in range(2):
                        for j in range(16):
                            ft = fpool.tile([P, W], bf16, tag="ft")
                            nc.scalar.activation(
                                ft[:], kp2[:, off:off + W], AF.Tanh,
                                bias=qp2[:, 32 * b + 16 * g + j:32 * b + 16 * g + j + 1],
                                scale=1.0)
                            for c0 in range(0, W, 512):
                                w = min(512, W - c0)
                                nc.tensor.matmul(sc[32 * g:32 * g + 32, c0:c0 + w],
                                                 lhsT=wvs_t[:, j, :], rhs=ft[:, c0:c0 + w],
                                                 start=(j == 0), stop=(j == 15))
                    pe = ppool.tile([QPC, W], bf16, tag="P")
                    nc.scalar.activation(pe[:], sc[:], AF.Exp)
                    # one PSUM bank holds all value accumulators:
                    # cols 0:64 = values[:,0:128].T @ P, 64:128 = values[:,128:256].T @ P,
                    # 128:192 (partitions 0:8) = [ones|pad].T @ P
                    vp = ps_v.tile([P, 3 * QPC], f32, tag="v")
                    for ci in range(nb):
                        tp = ps_tr.tile([P, QPC], bf16, tag="tr")
                        nc.tensor.transpose(tp[:], pe[:, CH * ci:CH * ci + CH],
                                            id_t[0:QPC, 0:QPC])
                        ptt = ptpool.tile([P, QPC], bf16, tag="pt")
                        nc.vector.tensor_copy(ptt[:], tp[:])
                        gi = off // CH + ci
                        # start=True clears has_written for the WHOLE bank, so
                        # only the very first matmul into this bank may use it;
                        # later first-writes rely on overwrite-where-bit-clear.
                        st, sp = (ci == 0), (ci == nb - 1)
                        nc.tensor.matmul(vp[:, 0:QPC], lhsT=va_t[:, gi, 0:128],
                                         rhs=ptt[:], start=st, stop=sp,
                                         skip_group_check=True)
                        nc.tensor.matmul(vp[:, QPC:2 * QPC], lhsT=va_t[:, gi, 128:256],
                                         rhs=ptt[:], start=False, stop=sp,
                                         skip_group_check=True)
                        nc.tensor.matmul(vp[0:8, 2 * QPC:3 * QPC],
                                         lhsT=va_t[:, gi, 256:264],
                                         rhs=ptt[:], start=False, stop=sp,
                                         skip_group_check=True)
                    # stream this region's whole accumulator out as one DMA
                    stg = ptpool.tile([P, 3 * QPC], f32, tag="stg")
                    nc.vector.tensor_copy(stg[:], vp[:])
                    nc.sync.dma_start(out_e[:, 3 * QPC * b:3 * QPC * (b + 1)], stg[:])

    nc.compile()
    return nc


def _prep_inputs(queries, keys, values, L, Wq, Wk, wv, nbs):
    import ml_dtypes
    bf = ml_dtypes.bfloat16
    C = sum(nbs) * CH
    keysT = np.zeros((D, C), np.float32)
    vaug = np.zeros((C, DPAD), np.float32)
    off = 0
    for b in range(B):
        nb = nbs[b]
        if nb == 0:
            continue
        n = nb * CH
        lb = int(L[b])
        kb = keys[b, :n, :].copy()
        vb = values[b, :n, :].copy()
        kb[lb:] = 0.0
        vb[lb:] = 0.0
        keysT[:, off:off + n] = kb.T
        vaug[off:off + n, 0:D] = vb
        vaug[off:off + n, D] = (np.arange(n) < lb).astype(np.float32)
        off += n
    wv2s = np.zeros((P, 16, 32), np.float32)
    for j in range(16):
        wv2s[0:H, j, 2 * j] = wv
        wv2s[H:P, j, 2 * j + 1] = wv
    wqk2 = np.concatenate([Wq, Wk, Wk], axis=1)            # (256, 192)
    wvsid = np.concatenate([wv2s.reshape(P, 512), np.eye(P, dtype=np.float32)],
                           axis=1)                          # (128, 640)
    keysT = keysT.astype(bf)
    vaug = vaug.astype(bf)
    wqk2 = np.ascontiguousarray(wqk2.astype(bf))
    wvsid = np.ascontiguousarray(wvsid.astype(bf))
    in_maps = []
    for c in range(NCORES):
        qT = np.zeros((D, 4 * QPC), np.float32)
        for b in range(B):
            qT[:, QPC * b:QPC * (b + 1)] = queries[b, c * QPC:(c + 1) * QPC, :].T
        in_maps.append(dict(qT=qT.astype(bf), keysT=keysT, vaug=vaug,
                            wqk2=wqk2, wvsid=wvsid))
    return in_maps


def kernel(queries, keys, values, valid_lens, Wq, Wk, wv):
    from concourse.bass_utils import run_bass_kernel_spmd

    queries = np.ascontiguousarray(np.asarray(queries, dtype=np.float32))
    keys = np.ascontiguousarray(np.asarray(keys, dtype=np.float32))
    values = np.ascontiguousarray(np.asarray(values, dtype=np.float32))
    Wq = np.ascontiguousarray(np.asarray(Wq, dtype=np.float32))
    Wk = np.ascontiguousarray(np.asarray(Wk, dtype=np.float32))
    wv = np.ascontiguousarray(np.asarray(wv, dtype=np.float32))
    L = np.clip(np.asarray(valid_lens).astype(np.int64), 0, KK)
    nbs = tuple(int(math.ceil(int(l) / CH)) for l in L)

    out = np.zeros((B, Q, D), np.float32)
    for b in range(B):
        if nbs[b] == 0:
            # softmax over all -1e6 scores is uniform over ALL keys
            out[b, :, :] = values[b].mean(axis=0)[None, :]

    if sum(nbs) > 0:
        if nbs not in _GRAPH_CACHE:
            _GRAPH_CACHE[nbs] = _build_graph(nbs)
        nc = _GRAPH_CACHE[nbs]
        in_maps = _prep_inputs(queries, keys, values, L, Wq, Wk, wv, nbs)
        res = run_bass_kernel_spmd(nc, in_maps, list(range(NCORES)))
        perm = np.array([32 * (r % 2) + r // 2 for r in range(QPC)])
        for c in range(NCORES):
            o = res.results[c]["out"]  # (128, 4*192): per region [d0|d1|sums]
            for b in range(B):
                if nbs[b] == 0:
                    continue
                blk = o[:, 3 * QPC * b:3 * QPC * (b + 1)]   # (128, 192)
                vals = np.concatenate([blk[:, 0:QPC], blk[:, QPC:2 * QPC]],
                                      axis=0)                # (256, 64)
                sums = blk[0, 2 * QPC:3 * QPC]               # (64,)
                out[b, c * QPC + perm, :] = (vals / sums[None, :]).T
    return out


# revision 19
# speedup vs baseline: 1.0676x; 1.0212x over previous
"""Additive attention (B=4, Q=512, K=2048, D=256, H=64) on 8 TRN2 NeuronCores.

Strategy
--------
scores[b,q,k] = sum_h wv[h] * tanh(qp[b,q,h] + kp[b,k,h]); masked softmax over k;
out = attn @ values.  The dominant cost is tanh on ACT (the only transcendental
engine, 1 elem/lane/cycle).  Structural optimizations:

1. Masked keys (k >= valid_len) contribute exp(-1e6) == 0 exactly, so only
   ceil(L_b/128)*128 key columns per batch need any compute.  The host gathers
   just the valid key-chunks; the graph is specialized at runtime to the
   valid_lens actually received (all 8 cores share one graph; per-core
   variation is data only).

2. Layout h-on-partitions with TWO queries packed per 128-partition ACT call
   (partitions 0:64 = h for q-slot-A, 64:128 = h for q-slot-B, against a kp2
   tile holding kp.T duplicated in both halves).  The "+qp" broadcast add is
   free via ACT's per-partition bias operand.  The h-reduction runs on the PE:
   pair j multiplies with a (128,32) weight whose only nonzero columns are 2j
   (top-half wv) and 2j+1 (bottom-half wv), accumulating into a shared PSUM
   tile -> 16 pairs build 32 score rows at no extra PE streaming cost.

3. All matmul operands are bf16 (fp32 streams at half rate on the PE; PSUM
   accumulation stays fp32).  ACT is dtype-independent so tanh/exp lose
   nothing; precision impact ~1e-3 total.

Softmax needs no max-subtraction (|score| <= ||wv||_1 ~ 2.6) and no on-device
normalization: an appended ones-column in the values matrix yields sum(exp)
via the same value matmul, and the host divides.  Each query lives on exactly
one core (core c owns q[64c:64c+64) of every batch), so no merge is needed.
"""

import math

import numpy as np

B, Q, KK, D, H = 4, 512, 2048, 256, 64
P = 128
CH = 128          # key-chunk granularity
QPC = 64          # queries per (core, batch)
NCORES = 8
DPAD = D + 8      # values cols + [ones, 7*zero] padding

_GRAPH_CACHE: dict = {}


def _build_graph(nbs):
    """Build + compile the single-core SPMD graph for per-batch chunk counts nbs."""
    import concourse.bass as bass
    import concourse.mybir as mybir
    from concourse import bacc
    from concourse.tile import TileContext

    f32 = mybir.dt.float32
    bf16 = mybir.dt.bfloat16
    AF = mybir.ActivationFunctionType
    C = sum(nbs) * CH
    assert C > 0
    offs = np.concatenate([[0], np.cumsum(np.asarray(nbs) * CH)]).astype(int)
    totch = C // CH

    nc = bacc.Bacc("TRN2", target_bir_lowering=False, debug=False)
    # wqk2: cols 0:64 = Wq, 64:192 = [Wk|Wk]; wvsid: cols 0:512 = wv2s flat,
    # 512:640 = identity.  Combined tensors keep the serialized per-DMA issue
    # cost (~0.6us each) off the critical path.
    qT_e = nc.declare_dram_parameter("qT", [D, 4 * QPC], bf16, isOutput=False)
    kT_e = nc.declare_dram_parameter("keysT", [D, C], bf16, isOutput=False)
    va_e = nc.declare_dram_parameter("vaug", [C, DPAD], bf16, isOutput=False)
    wqk2_e = nc.declare_dram_parameter("wqk2", [D, H + P], bf16, isOutput=False)
    wvsid_e = nc.declare_dram_parameter("wvsid", [P, 16 * 32 + P], bf16,
                                        isOutput=False)
    out_e = nc.declare_dram_parameter("out", [P, 4 * 3 * QPC], f32, isOutput=True)

    with TileContext(nc) as tc:
        with (
            tc.tile_pool(name="const", bufs=1) as cpool,
            tc.tile_pool(name="big", bufs=1) as kpool,
            tc.tile_pool(name="feat", bufs=6) as fpool,
            tc.tile_pool(name="pexp", bufs=2) as ppool,
            tc.tile_pool(name="pts", bufs=4) as ptpool,
        ):
            # ---- input loads, critical-path first
            wqk2_t = cpool.tile([P, 2, H + P], bf16)
            nc.sync.dma_start(wqk2_t[:], wqk2_e[:].rearrange("(c p) h -> p c h", p=P))
            qt_t = cpool.tile([P, 2, 4 * QPC], bf16)
            nc.sync.dma_start(qt_t[:], qT_e[:].rearrange("(c p) q -> p c q", p=P))
            # keysT in two tiles: first 512 cols land fast so kp2 chunk 0 (and
            # the first tanh) start early; the rest follows as one big DMA.
            w1 = min(512, C)
            kt_a = kpool.tile([P, 2, w1], bf16)
            nc.sync.dma_start(kt_a[:], kT_e[:, 0:w1].rearrange("(c p) k -> p c k", p=P))
            kt_b = None
            if C > w1:
                kt_b = kpool.tile([P, 2, C - w1], bf16)
                nc.sync.dma_start(kt_b[:],
                                  kT_e[:, w1:C].rearrange("(c p) k -> p c k", p=P))
            wvs_t = cpool.tile([P, 16, 32], bf16)
            nc.sync.dma_start(wvs_t[:], wvsid_e[:, 0:512].rearrange(
                "p (j c) -> p j c", j=16))
            id_t = cpool.tile([P, P], bf16)
            nc.sync.dma_start(id_t[:], wvsid_e[:, 512:512 + P])
            va_t = kpool.tile([P, totch, DPAD], bf16)
            nc.sync.dma_start(va_t[:], va_e[:].rearrange("(n p) d -> p n d", p=P))

            # ---- kp2 = Wk2.T @ keysT  -> (128, C) in SBUF (both halves = kp.T)
            kp2 = kpool.tile([P, C], bf16)
            qp2 = cpool.tile([P, QPC // 2 * 4], f32)
            with tc.tile_pool(name="ps_mm", bufs=2, space="PSUM") as ps_mm:
                # qp2 first: tiny DMA + matmuls; bias must be ready for tanh #1
                qps = ps_mm.tile([H, 4 * QPC], f32, tag="qp")
                nc.tensor.matmul(qps[:], lhsT=wqk2_t[:, 0, 0:H], rhs=qt_t[:, 0, :],
                                 start=True, stop=False)
                nc.tensor.matmul(qps[:], lhsT=wqk2_t[:, 1, 0:H], rhs=qt_t[:, 1, :],
                                 start=False, stop=True)
                for b in range(B):
                    if nbs[b] == 0:
                        continue
                    nc.vector.tensor_copy(qp2[0:H, 32 * b:32 * b + 32],
                                          qps[:, QPC * b:QPC * b + 32])
                    nc.vector.tensor_copy(qp2[H:P, 32 * b:32 * b + 32],
                                          qps[:, QPC * b + 32:QPC * b + 64])

                for c0 in range(0, C, 512):
                    w = min(512, C - c0)
                    if c0 < w1:
                        r0, r1 = kt_a[:, 0, c0:c0 + w], kt_a[:, 1, c0:c0 + w]
                    else:
                        r0 = kt_b[:, 0, c0 - w1:c0 - w1 + w]
                        r1 = kt_b[:, 1, c0 - w1:c0 - w1 + w]
                    pt = ps_mm.tile([P, 512], f32, tag="kp")
                    nc.tensor.matmul(pt[:, :w], lhsT=wqk2_t[:, 0, H:H + P],
                                     rhs=r0, start=True, stop=False)
                    nc.tensor.matmul(pt[:, :w], lhsT=wqk2_t[:, 1, H:H + P],
                                     rhs=r1, start=False, stop=True)
                    nc.vector.tensor_copy(kp2[:, c0:c0 + w], pt[:, :w])

            # ---- per-batch regions
            with (
                tc.tile_pool(name="ps_sc", bufs=1, space="PSUM") as ps_sc,
                tc.tile_pool(name="ps_tr", bufs=2, space="PSUM") as ps_tr,
                tc.tile_pool(name="ps_v", bufs=1, space="PSUM") as ps_v,
            ):
                for b in range(B):
                    nb = nbs[b]
                    if nb == 0:
                        continue
                    W = nb * CH
                    off = int(offs[b])
                    sc = ps_sc.tile([QPC, W], f32, tag="sc")
                    for g in range(2):
                        for j in range(16):
                            ft = fpool.tile([P, W], bf16, tag="ft")
                            nc.scalar.activation(
                                ft[:], kp2[:, off:off + W], AF.Tanh,
                                bias=qp2[:, 32 * b + 16 * g + j:32 * b + 16 * g + j + 1],
                                scale=1.0)
                            for c0 in range(0, W, 512):
                                w = min(512, W - c0)
                                nc.tensor.matmul(sc[32 * g:32 * g + 32, c0:c0 + w],
                                                 lhsT=wvs_t[:, j, :], rhs=ft[:, c0:c0 + w],
                                                 start=(j == 0), stop=(j == 15))
                    pe = ppool.tile([QPC, W], bf16, tag="P")
                    nc.scalar.activation(pe[:], sc[:], AF.Exp)
                    # one PSUM bank holds all value accumulators:
                    # cols 0:64 = values[:,0:128].T @ P, 64:128 = values[:,128:256].T @ P,
                    # 128:192 (partitions 0:8) = [ones|pad].T @ P
                    vp = ps_v.tile([P, 3 * QPC], f32, tag="v")
                    for ci in range(nb):
                        tp = ps_tr.tile([P, QPC], bf16, tag="tr")
                        nc.tensor.transpose(tp[:], pe[:, CH * ci:CH * ci + CH],
                                            id_t[0:QPC, 0:QPC])
                        ptt = ptpool.tile([P, QPC], bf16, tag="pt")
                        nc.vector.tensor_copy(ptt[:], tp[:])
                        gi = off // CH + ci
                        # start=True clears has_written for the WHOLE bank, so
                        # only the very first matmul into this bank may use it;
                        # later first-writes rely on overwrite-where-bit-clear.
                        st, sp = (ci == 0), (ci == nb - 1)
                        nc.tensor.matmul(vp[:, 0:QPC], lhsT=va_t[:, gi, 0:128],
                                         rhs=ptt[:], start=st, stop=sp,
                                         skip_group_check=True)
                        nc.tensor.matmul(vp[:, QPC:2 * QPC], lhsT=va_t[:, gi, 128:256],
                                         rhs=ptt[:], start=False, stop=sp,
                                         skip_group_check=True)
                        nc.tensor.matmul(vp[0:8, 2 * QPC:3 * QPC],
                                         lhsT=va_t[:, gi, 256:264],
                                         rhs=ptt[:], start=False, stop=sp,
                                         skip_group_check=True)
                    # stream this region's whole accumulator out as one DMA
                    stg = ptpool.tile([P, 3 * QPC], f32, tag="stg")
                    nc.vector.tensor_copy(stg[:], vp[:])
                    nc.sync.dma_start(out_e[:, 3 * QPC * b:3 * QPC * (b + 1)], stg[:])

    nc.compile()
    return nc


def _prep_inputs(queries, keys, values, L, Wq, Wk, wv, nbs):
    import ml_dtypes
    bf = ml_dtypes.bfloat16
    C = sum(nbs) * CH
    keysT = np.zeros((D, C), np.float32)
    vaug = np.zeros((C, DPAD), np.float32)
    off = 0
    for b in range(B):
        nb = nbs[b]
        if nb == 0:
            continue
        n = nb * CH
        lb = int(L[b])
        kb = keys[b, :n, :].copy()
        vb = values[b, :n, :].copy()
        kb[lb:] = 0.0
        vb[lb:] = 0.0
        keysT[:, off:off + n] = kb.T
        vaug[off:off + n, 0:D] = vb
        vaug[off:off + n, D] = (np.arange(n) < lb).astype(np.float32)
        off += n
    wv2s = np.zeros((P, 16, 32), np.float32)
    for j in range(16):
        wv2s[0:H, j, 2 * j] = wv
        wv2s[H:P, j, 2 * j + 1] = wv
    wqk2 = np.concatenate([Wq, Wk, Wk], axis=1)            # (256, 192)
    wvsid = np.concatenate([wv2s.reshape(P, 512), np.eye(P, dtype=np.float32)],
                           axis=1)                          # (128, 640)
    keysT = keysT.astype(bf)
    vaug = vaug.astype(bf)
    wqk2 = np.ascontiguousarray(wqk2.astype(bf))
    wvsid = np.ascontiguousarray(wvsid.astype(bf))
    in_maps = []
    for c in range(NCORES):
        qT = np.zeros((D, 4 * QPC), np.float32)
        for b in range(B):
            qT[:, QPC * b:QPC * (b + 1)] = queries[b, c * QPC:(c + 1) * QPC, :].T
        in_maps.append(dict(qT=qT.astype(bf), keysT=keysT, vaug=vaug,
                            wqk2=wqk2, wvsid=wvsid))
    return in_maps


def kernel(queries, keys, values, valid_lens, Wq, Wk, wv):
    from concourse.bass_utils import run_bass_kernel_spmd

    queries = np.ascontiguousarray(np.asarray(queries, dtype=np.float32))
    keys = np.ascontiguousarray(np.asarray(keys, dtype=np.float32))
    values = np.ascontiguousarray(np.asarray(values, dtype=np.float32))
    Wq = np.ascontiguousarray(np.asarray(Wq, dtype=np.float32))
    Wk = np.ascontiguousarray(np.asarray(Wk, dtype=np.float32))
    wv = np.ascontiguousarray(np.asarray(wv, dtype=np.float32))
    L = np.clip(np.asarray(valid_lens).astype(np.int64), 0, KK)
    nbs = tuple(int(math.ceil(int(l) / CH)) for l in L)

    out = np.zeros((B, Q, D), np.float32)
    for b in range(B):
        if nbs[b] == 0:
            # softmax over all -1e6 scores is uniform over ALL keys
            out[b, :, :] = values[b].mean(axis=0)[None, :]

    if sum(nbs) > 0:
        if nbs not in _GRAPH_CACHE:
            _GRAPH_CACHE[nbs] = _build_graph(nbs)
        nc = _GRAPH_CACHE[nbs]
        in_maps = _prep_inputs(queries, keys, values, L, Wq, Wk, wv, nbs)
        res = run_bass_kernel_spmd(nc, in_maps, list(range(NCORES)))
        perm = np.array([32 * (r % 2) + r // 2 for r in range(QPC)])
        for c in range(NCORES):
            o = res.results[c]["out"]  # (128, 4*192): per region [d0|d1|sums]
            for b in range(B):
                if nbs[b] == 0:
                    continue
                blk = o[:, 3 * QPC * b:3 * QPC * (b + 1)]   # (128, 192)
                vals = np.concatenate([blk[:, 0:QPC], blk[:, QPC:2 * QPC]],
                                      axis=0)                # (256, 64)
                sums = blk[0, 2 * QPC:3 * QPC]               # (64,)
                out[b, c * QPC + perm, :] = (vals / sums[None, :]).T
    return out


# revision 21
# speedup vs baseline: 1.0852x; 1.0165x over previous
"""Additive attention (B=4, Q=512, K=2048, D=256, H=64) on 8 TRN2 NeuronCores.

Strategy
--------
scores[b,q,k] = sum_h wv[h] * tanh(qp[b,q,h] + kp[b,k,h]); masked softmax over k;
out = attn @ values.  The dominant cost is tanh on ACT (the only transcendental
engine, 1 elem/lane/cycle).  Structural optimizations:

1. Masked keys (k >= valid_len) contribute exp(-1e6) == 0 exactly, so only
   ceil(L_b/128)*128 key columns per batch need any compute.  The host gathers
   just the valid key-chunks; the graph is specialized at runtime to the
   valid_lens actually received (all 8 cores share one graph; per-core
   variation is data only).

2. Layout h-on-partitions with TWO queries packed per 128-partition ACT call
   (partitions 0:64 = h for q-slot-A, 64:128 = h for q-slot-B, against a kp2
   tile holding kp.T duplicated in both halves).  The "+qp" broadcast add is
   free via ACT's per-partition bias operand.  The h-reduction runs on the PE:
   pair j multiplies with a (128,32) weight whose only nonzero columns are 2j
   (top-half wv) and 2j+1 (bottom-half wv), accumulating into a shared PSUM
   tile -> 16 pairs build 32 score rows at no extra PE streaming cost.

3. All matmul operands are bf16 (fp32 streams at half rate on the PE; PSUM
   accumulation stays fp32).  ACT is dtype-independent so tanh/exp lose
   nothing; precision impact ~1e-3 total.

Softmax needs no max-subtraction (|score| <= ||wv||_1 ~ 2.6) and no on-device
normalization: an appended ones-column in the values matrix yields sum(exp)
via the same value matmul, and the host divides.  Each query lives on exactly
one core (core c owns q[64c:64c+64) of every batch), so no merge is needed.
"""

import math

import numpy as np

B, Q, KK, D, H = 4, 512, 2048, 256, 64
P = 128
CH = 128          # key-chunk granularity
QPC = 64          # queries per (core, batch)
NCORES = 8
DPAD = D + 8      # values cols + [ones, 7*zero] padding

_GRAPH_CACHE: dict = {}


def _build_graph(nbs):
    """Build + compile the single-core SPMD graph for per-batch chunk counts nbs."""
    import concourse.bass as bass
    import concourse.mybir as mybir
    from concourse import bacc
    from concourse.tile import TileContext

    f32 = mybir.dt.float32
    bf16 = mybir.dt.bfloat16
    AF = mybir.ActivationFunctionType
    C = sum(nbs) * CH
    assert C > 0
    offs = np.concatenate([[0], np.cumsum(np.asarray(nbs) * CH)]).astype(int)
    totch = C // CH

    nc = bacc.Bacc("TRN2", target_bir_lowering=False, debug=False)
    # wqk2: cols 0:64 = Wq, 64:192 = [Wk|Wk]; wvsid: cols 0:512 = wv2s flat,
    # 512:640 = identity.  Combined tensors keep the serialized per-DMA issue
    # cost (~0.6us each) off the critical path.
    qT_e = nc.declare_dram_parameter("qT", [D, 4 * QPC], bf16, isOutput=False)
    kT_e = nc.declare_dram_parameter("keysT", [D, C], bf16, isOutput=False)
    va_e = nc.declare_dram_parameter("vaug", [C, DPAD], bf16, isOutput=False)
    wqk2_e = nc.declare_dram_parameter("wqk2", [D, H + P], bf16, isOutput=False)
    wvsid_e = nc.declare_dram_parameter("wvsid", [P, 16 * 32 + P], bf16,
                                        isOutput=False)
    out_e = nc.declare_dram_parameter("out", [P, 4 * 3 * QPC], f32, isOutput=True)

    with TileContext(nc) as tc:
        with (
            tc.tile_pool(name="const", bufs=1) as cpool,
            tc.tile_pool(name="big", bufs=1) as kpool,
            tc.tile_pool(name="feat", bufs=6) as fpool,
            tc.tile_pool(name="pexp", bufs=2) as ppool,
            tc.tile_pool(name="pts", bufs=4) as ptpool,
        ):
            # ---- input loads, critical-path first
            wqk2_t = cpool.tile([P, 2, H + P], bf16)
            nc.sync.dma_start(wqk2_t[:], wqk2_e[:].rearrange("(c p) h -> p c h", p=P))
            # keysT in two tiles: first 512 cols land fast so kp2 chunk 0 (and
            # the first tanh) start early; the rest follows as one big DMA.
            w1 = min(512, C)
            kt_a = kpool.tile([P, 2, w1], bf16)
            nc.sync.dma_start(kt_a[:], kT_e[:, 0:w1].rearrange("(c p) k -> p c k", p=P))
            qt_t = cpool.tile([P, 2, 4 * QPC], bf16)
            nc.sync.dma_start(qt_t[:], qT_e[:].rearrange("(c p) q -> p c q", p=P))
            wvs_t = cpool.tile([P, 16, 32], bf16)
            nc.sync.dma_start(wvs_t[:], wvsid_e[:, 0:512].rearrange(
                "p (j c) -> p j c", j=16))
            kt_b = None
            if C > w1:
                kt_b = kpool.tile([P, 2, C - w1], bf16)
                nc.sync.dma_start(kt_b[:],
                                  kT_e[:, w1:C].rearrange("(c p) k -> p c k", p=P))
            id_t = cpool.tile([P, P], bf16)
            nc.sync.dma_start(id_t[:], wvsid_e[:, 512:512 + P])
            va_t = kpool.tile([P, totch, DPAD], bf16)
            nc.sync.dma_start(va_t[:], va_e[:].rearrange("(n p) d -> p n d", p=P))

            # ---- kp2 = Wk2.T @ keysT  -> (128, C) in SBUF (both halves = kp.T)
            # 512-col slices are emitted lazily, right before the first region
            # that reads them, so PE score matmuls are not queued behind the
            # whole projection.
            kp2 = kpool.tile([P, C], bf16)
            qp2 = cpool.tile([P, QPC // 2 * 4], f32)

            def emit_kp2_chunk(ps_pool, c0):
                w = min(512, C - c0)
                if c0 < w1:
                    r0, r1 = kt_a[:, 0, c0:c0 + w], kt_a[:, 1, c0:c0 + w]
                else:
                    r0 = kt_b[:, 0, c0 - w1:c0 - w1 + w]
                    r1 = kt_b[:, 1, c0 - w1:c0 - w1 + w]
                pt = ps_pool.tile([P, 512], f32, tag="kp")
                nc.tensor.matmul(pt[:, :w], lhsT=wqk2_t[:, 0, H:H + P],
                                 rhs=r0, start=True, stop=False)
                nc.tensor.matmul(pt[:, :w], lhsT=wqk2_t[:, 1, H:H + P],
                                 rhs=r1, start=False, stop=True)
                nc.vector.tensor_copy(kp2[:, c0:c0 + w], pt[:, :w])

            # ---- per-batch regions
            with (
                tc.tile_pool(name="ps_mm", bufs=2, space="PSUM") as ps_mm,
                tc.tile_pool(name="ps_sc", bufs=1, space="PSUM") as ps_sc,
                tc.tile_pool(name="ps_tr", bufs=1, space="PSUM") as ps_tr,
                tc.tile_pool(name="ps_v", bufs=1, space="PSUM") as ps_v,
            ):
                emit_kp2_chunk(ps_mm, 0)

                # qp2 bias tile; pair j of batch b = (q_{64b+j}, q_{64b+32+j});
                # two strided copies build all four batch blocks at once
                qps = ps_mm.tile([H, 4 * QPC], f32, tag="kp")
                nc.tensor.matmul(qps[:], lhsT=wqk2_t[:, 0, 0:H], rhs=qt_t[:, 0, :],
                                 start=True, stop=False)
                nc.tensor.matmul(qps[:], lhsT=wqk2_t[:, 1, 0:H], rhs=qt_t[:, 1, :],
                                 start=False, stop=True)
                qps_r = qps[:].rearrange("h (b c) -> h b c", b=B)
                qp2_r = qp2[:].rearrange("p (b c) -> p b c", b=B)
                nc.vector.tensor_copy(qp2_r[0:H], qps_r[:, :, 0:32])
                nc.vector.tensor_copy(qp2_r[H:P], qps_r[:, :, 32:QPC])

                emitted = 512
                for b in range(B):
                    nb = nbs[b]
                    if nb == 0:
                        continue
                    W = nb * CH
                    off = int(offs[b])
                    while emitted < min(off + W, C):
                        emit_kp2_chunk(ps_mm, emitted)
                        emitted += 512
                    sc = ps_sc.tile([QPC, W], f32, tag="sc")
                    for g in range(2):
                        for j in range(16):
                            ft = fpool.tile([P, W], bf16, tag="ft")
                            nc.scalar.activation(
                                ft[:], kp2[:, off:off + W], AF.Tanh,
                                bias=qp2[:, 32 * b + 16 * g + j:32 * b + 16 * g + j + 1],
                                scale=1.0)
                            for c0 in range(0, W, 512):
                                w = min(512, W - c0)
                                nc.tensor.matmul(sc[32 * g:32 * g + 32, c0:c0 + w],
                                                 lhsT=wvs_t[:, j, :], rhs=ft[:, c0:c0 + w],
                                                 start=(j == 0), stop=(j == 15))
                    pe = ppool.tile([QPC, W], bf16, tag="P")
                    nc.scalar.activation(pe[:], sc[:], AF.Exp)
                    # one PSUM bank holds all value accumulators:
                    # cols 0:64 = values[:,0:128].T @ P, 64:128 = values[:,128:256].T @ P,
                    # 128:192 (partitions 0:8) = [ones|pad].T @ P
                    vp = ps_v.tile([P, 3 * QPC], f32, tag="v")
                    for ci in range(nb):
                        tp = ps_tr.tile([P, QPC], bf16, tag="tr")
                        nc.tensor.transpose(tp[:], pe[:, CH * ci:CH * ci + CH],
                                            id_t[0:QPC, 0:QPC])
                        ptt = ptpool.tile([P, QPC], bf16, tag="pt")
                        nc.vector.tensor_copy(ptt[:], tp[:])
                        gi = off // CH + ci
                        # start=True clears has_written for the WHOLE bank, so
                        # only the very first matmul into this bank may use it;
                        # later first-writes rely on overwrite-where-bit-clear.
                        st, sp = (ci == 0), (ci == nb - 1)
                        nc.tensor.matmul(vp[:, 0:QPC], lhsT=va_t[:, gi, 0:128],
                                         rhs=ptt[:], start=st, stop=sp,
                                         skip_group_check=True)
                        nc.tensor.matmul(vp[:, QPC:2 * QPC], lhsT=va_t[:, gi, 128:256],
                                         rhs=ptt[:], start=False, stop=sp,
                                         skip_group_check=True)
                        nc.tensor.matmul(vp[0:8, 2 * QPC:3 * QPC],
                                         lhsT=va_t[:, gi, 256:264],
                                         rhs=ptt[:], start=False, stop=sp,
                                         skip_group_check=True)
                    # stream this region's whole accumulator out as one DMA
                    stg = ptpool.tile([P, 3 * QPC], f32, tag="stg")
                    nc.vector.tensor_copy(stg[:], vp[:])
                    nc.sync.dma_start(out_e[:, 3 * QPC * b:3 * QPC * (b + 1)], stg[:])

    nc.compile()
    return nc


def _prep_inputs(queries, keys, values, L, Wq, Wk, wv, nbs):
    import ml_dtypes
    bf = ml_dtypes.bfloat16
    C = sum(nbs) * CH
    keysT = np.zeros((D, C), np.float32)
    vaug = np.zeros((C, DPAD), np.float32)
    off = 0
    for b in range(B):
        nb = nbs[b]
        if nb == 0:
            continue
        n = nb * CH
        lb = int(L[b])
        kb = keys[b, :n, :].copy()
        vb = values[b, :n, :].copy()
        kb[lb:] = 0.0
        vb[lb:] = 0.0
        keysT[:, off:off + n] = kb.T
        vaug[off:off + n, 0:D] = vb
        vaug[off:off + n, D] = (np.arange(n) < lb).astype(np.float32)
        off += n
    wv2s = np.zeros((P, 16, 32), np.float32)
    for j in range(16):
        wv2s[0:H, j, 2 * j] = wv
        wv2s[H:P, j, 2 * j + 1] = wv
    wqk2 = np.concatenate([Wq, Wk, Wk], axis=1)            # (256, 192)
    wvsid = np.concatenate([wv2s.reshape(P, 512), np.eye(P, dtype=np.float32)],
                           axis=1)                          # (128, 640)
    keysT = keysT.astype(bf)
    vaug = vaug.astype(bf)
    wqk2 = np.ascontiguousarray(wqk2.astype(bf))
    wvsid = np.ascontiguousarray(wvsid.astype(bf))
    in_maps = []
    for c in range(NCORES):
        qT = np.zeros((D, 4 * QPC), np.float32)
        for b in range(B):
            qT[:, QPC * b:QPC * (b + 1)] = queries[b, c * QPC:(c + 1) * QPC, :].T
        in_maps.append(dict(qT=qT.astype(bf), keysT=keysT, vaug=vaug,
                            wqk2=wqk2, wvsid=wvsid))
    return in_maps


def kernel(queries, keys, values, valid_lens, Wq, Wk, wv):
    from concourse.bass_utils import run_bass_kernel_spmd

    queries = np.ascontiguousarray(np.asarray(queries, dtype=np.float32))
    keys = np.ascontiguousarray(np.asarray(keys, dtype=np.float32))
    values = np.ascontiguousarray(np.asarray(values, dtype=np.float32))
    Wq = np.ascontiguousarray(np.asarray(Wq, dtype=np.float32))
    Wk = np.ascontiguousarray(np.asarray(Wk, dtype=np.float32))
    wv = np.ascontiguousarray(np.asarray(wv, dtype=np.float32))
    L = np.clip(np.asarray(valid_lens).astype(np.int64), 0, KK)
    nbs = tuple(int(math.ceil(int(l) / CH)) for l in L)

    out = np.zeros((B, Q, D), np.float32)
    for b in range(B):
        if nbs[b] == 0:
            # softmax over all -1e6 scores is uniform over ALL keys
            out[b, :, :] = values[b].mean(axis=0)[None, :]

    if sum(nbs) > 0:
        if nbs not in _GRAPH_CACHE:
            _GRAPH_CACHE[nbs] = _build_graph(nbs)
        nc = _GRAPH_CACHE[nbs]
        in_maps = _prep_inputs(queries, keys, values, L, Wq, Wk, wv, nbs)
        res = run_bass_kernel_spmd(nc, in_maps, list(range(NCORES)))
        perm = np.array([32 * (r % 2) + r // 2 for r in range(QPC)])
        for c in range(NCORES):
            o = res.results[c]["out"]  # (128, 4*192): per region [d0|d1|sums]
            for b in range(B):
                if nbs[b] == 0:
                    continue
                blk = o[:, 3 * QPC * b:3 * QPC * (b + 1)]   # (128, 192)
                vals = np.concatenate([blk[:, 0:QPC], blk[:, QPC:2 * QPC]],
                                      axis=0)                # (256, 64)
                sums = blk[0, 2 * QPC:3 * QPC]               # (64,)
                out[b, c * QPC + perm, :] = (vals / sums[None, :]).T
    return out
